# revision 32
# baseline (speedup 1.0000x reference)
"""HeadQK kernel for trn2: out = segsum_vocab(causal(q @ k.T / 256)) over 8 cores.

Strategy: cover the causally-active (j-tile, t-chunk) blocks of the T x T
attention matrix c with 8 uniform regions, one per core.  Each region is
NQ=3 t-chunks x NK=8 j-tiles (24 blocks); a block computes
c[j, t] = <k_j, q_t> with k = x @ Wk, q = x @ (Wq/256).  A core computes q
only for its region's 3 chunks and k only for its 8 j-tiles, so the big
projection work is split across cores instead of replicated.  The
regions are an exact cover (solver-verified) with the extra property
tiles[0:4] == T(chunks[0]): the k half-0 rows ARE the first q-chunk's
rows, so that 1MB of x is transferred once and read by both passes
(5MB input per core instead of 6.3MB).

Scheduling notes (all matmuls are [128,128]x[128,512] bf16 accumulation
chains at the ~213ns back-to-back PE issue rate; switching PSUM banks
between matmuls costs ~46ns, so every accumulation runs as one
contiguous chain into one bank):
 - inputs stream in need-ordered pieces over BOTH HWDGE groups (SP
   carries the xh stream that gates k-h0/q, Activation carries
   wk/wq/xk1 in parallel) so the shared ~0.4MB/us HBM read rate is
   always spent on the next tensor the PE needs;
 - chunk-0's xh arrives as a 3-piece ladder so the first chain starts
   as early as the ~2.5us DMA doorbell->data latency allows;
 - compute order k-h0, q-i0 (zero new bytes), k-h1, q-i1, q-i2, then
   the input-free c blocks;
 - all PSUM tiles share one 8-bank ring pool; PSUM->SBUF casts
   alternate vector/scalar;
 - c blocks for one j-tile go out as a single contiguous-line
   [128,1536] DMA on SP; the last tile is split per-block so the final
   issue trails the final matmul by only ~1.5us.
The host applies the causal tril mask and the vocab segment-sum in fp32.
The device program is identical on every core (SPMD); per-core work
differs only through input data.
"""

import sys

import numpy as np

if "/opt/trn_rl_repo" not in sys.path:
    sys.path.insert(0, "/opt/trn_rl_repo")

import ml_dtypes

import concourse.bacc as bacc
import concourse.mybir as mybir
import concourse.tile as tile
from concourse.bass_utils import run_bass_kernel_spmd

T, C, D, V = 4096, 1024, 256, 32000
NCORES = 8
NCH = 8            # t chunks in T
CW = T // NCH      # 512
NQ = 3             # t-chunks per region
NK = 8             # j-tiles per region
HK = NK // 2       # j-tiles per xkt half
CT = C // 128      # 8 contraction tiles
DT = D // 128      # 2 d tiles
F32 = mybir.dt.float32
BF16 = mybir.dt.bfloat16
BF = ml_dtypes.bfloat16

# core p computes blocks (g, ch) for ch in REGIONS[p][0], g in REGIONS[p][1];
# together the regions cover every causally-active block (ch >= g//4).
REGIONS = [
    ([3, 4, 5], [12, 13, 14, 15, 0, 1, 2, 3]),
    ([0, 6, 7], [0, 1, 2, 3, 12, 13, 14, 15]),
    ([1, 2, 3], [4, 5, 6, 7, 0, 1, 2, 3]),
    ([4, 5, 6], [16, 17, 18, 19, 4, 5, 6, 7]),
    ([5, 4, 6], [20, 21, 22, 23, 8, 9, 10, 11]),
    ([2, 3, 7], [8, 9, 10, 11, 4, 5, 6, 7]),
    ([5, 6, 7], [20, 21, 22, 23, 24, 25, 26, 27]),
    ([4, 0, 7], [16, 17, 18, 19, 28, 29, 30, 31]),
]

# input piece tables: (name, c8_lo, c8_hi).  The k half-0 j-tiles are the
# rows of the core's FIRST q-chunk (tiles[0:4] == T(chunks[0])), so that x
# data is read from the xh0 buffers and never transferred twice; only the
# half-1 free tiles arrive as dedicated xk tensors.
WK_PIECES = (("wk0", 0, 2), ("wk1", 2, 8))
XK1_PIECES = (("xk1p0", 0, 4), ("xk1p1", 4, 8))
XH0_PIECES = (("xh0p0", 0, 2), ("xh0p1", 2, 4), ("xh0p2", 4, 8))


def _build():
    nc = bacc.Bacc("TRN2", target_bir_lowering=False, debug=False,
                   num_devices=NCORES)
    dram = {}
    for nm, lo, hi in WK_PIECES:
        dram[nm] = nc.dram_tensor(nm, [128, (hi - lo) * 256], BF16,
                                  kind="ExternalInput")
    for nm, lo, hi in XK1_PIECES:
        dram[nm] = nc.dram_tensor(nm, [128, (hi - lo) * CW], BF16,
                                  kind="ExternalInput")
    dram["wq"] = nc.dram_tensor("wq", [128, CT * 256], BF16,
                                kind="ExternalInput")
    for nm, lo, hi in XH0_PIECES:
        dram[nm] = nc.dram_tensor(nm, [128, (hi - lo) * CW], BF16,
                                  kind="ExternalInput")
    for i in range(1, NQ):
        for s in range(2):
            nm = f"xh{i}{s}"
            dram[nm] = nc.dram_tensor(nm, [128, 4 * CW], BF16,
                                      kind="ExternalInput")
    out = nc.dram_tensor("out", [NK, 128, NQ * CW], BF16,
                         kind="ExternalOutput")

    with tile.TileContext(nc) as tc:
        with (
            tc.tile_pool(name="const", bufs=1) as cpool,
            tc.tile_pool(name="obuf", bufs=4) as opool,
            tc.tile_pool(name="psall", bufs=7, space="PSUM") as psqk,
            tc.tile_pool(name="pswarm", bufs=1, space="PSUM") as pswarm,
        ):
            sb = {}
            for nm in dram:
                if nm == "out":
                    continue
                sb[nm] = cpool.tile(list(dram[nm].shape), BF16, tag=nm,
                                    name=f"{nm}b")
            # ALL input DMAs on the SP group in strict need order: HBM
            # read bandwidth (~0.4MB/us) is shared across queue groups, so
            # a second group only dilutes the gate's priority -- within one
            # group the queues drain DMAs strictly in issue order.
            for nm in ("wk0", "xh0p0", "wk1", "xh0p1", "xh0p2",
                       "wq", "xk1p0", "xk1p1",
                       "xh10", "xh11", "xh20", "xh21"):
                nc.sync.dma_start(out=sb[nm][:], in_=dram[nm][:])

            # PE warm-up: the power manager ramps the PE from half rate
            # over its first ~8-10us of activity.  The PE is idle waiting
            # for the first input anyway, so burn the ramp on dummy
            # matmuls over zeroed scratch (results never read).
            warm_sb = cpool.tile([128, CW], BF16, tag="warm", name="warm_sb")
            nc.gpsimd.memset(warm_sb[:], 0.0)
            wp = pswarm.tile([128, CW], F32, tag="wp", name="wp")
            for _ in range(10):
                nc.tensor.matmul(out=wp[:], lhsT=warm_sb[:, 0:128],
                                 rhs=warm_sb[:], start=True, stop=True)

            def wk_slice(c8, d):
                nm, lo, _ = next(p for p in WK_PIECES if p[1] <= c8 < p[2])
                o = (c8 - lo) * 256 + d * 128
                return sb[nm][:, o:o + 128]

            def xk_slice(h, c8):
                if h == 0:
                    return xh_slice(0, c8)
                nm, lo, _ = next(p for p in XK1_PIECES if p[1] <= c8 < p[2])
                o = (c8 - lo) * CW
                return sb[nm][:, o:o + CW]

            def xh_slice(i, c8):
                if i == 0:
                    nm, lo, _ = next(p for p in XH0_PIECES
                                     if p[1] <= c8 < p[2])
                    return sb[nm][:, (c8 - lo) * CW:(c8 - lo + 1) * CW]
                o = (c8 % 4) * CW
                return sb[f"xh{i}{c8 // 4}"][:, o:o + CW]

            # cast engine round-robin: vector / scalar
            ncast = [0]

            def psum_to_sbuf(dst, src):
                if ncast[0] % 2 == 0:
                    nc.vector.tensor_copy(out=dst, in_=src)
                else:
                    nc.scalar.copy(out=dst, in_=src)
                ncast[0] += 1

            # --- compute schedule: projection chains in input-stream
            # order (k halves, then q chunks), then the input-free c
            # blocks; every accumulation is one contiguous PE chain.
            # ktb[d][dp, tt*128 + jj] = k[j-tile tt, j=jj, d*128 + dp]
            ktb = [cpool.tile([128, NK * 128], BF16, tag=f"kt{d}",
                              name=f"ktb{d}") for d in range(DT)]
            qt = [cpool.tile([128, NQ * CW], BF16, tag=f"qt{d}",
                             name=f"qtb{d}") for d in range(DT)]

            def k_half(h, interleave=False):
                # interleave=True: c8-outer, both d accumulators open, so
                # the PE consumes the x stream at DMA pace during the gate
                kp = [psqk.tile([128, CW], F32, tag="ps", name="kp")
                      for _ in range(DT)]
                order = ([(c8, d) for c8 in range(CT) for d in range(DT)]
                         if interleave else
                         [(c8, d) for d in range(DT) for c8 in range(CT)])
                for c8, d in order:
                    nc.tensor.matmul(
                        out=kp[d][:], lhsT=wk_slice(c8, d),
                        rhs=xk_slice(h, c8),
                        start=(c8 == 0), stop=(c8 == CT - 1),
                    )
                for d in range(DT):
                    psum_to_sbuf(ktb[d][:, h * CW:(h + 1) * CW], kp[d][:])

            def q_chunk(i, interleave=False):
                qp = [psqk.tile([128, CW], F32, tag="ps", name="qp")
                      for _ in range(DT)]
                order = ([(c8, d) for c8 in range(CT) for d in range(DT)]
                         if interleave else
                         [(c8, d) for d in range(DT) for c8 in range(CT)])
                for c8, d in order:
                    nc.tensor.matmul(
                        out=qp[d][:],
                        lhsT=sb["wq"][:, c8 * 256 + d * 128:
                                      c8 * 256 + (d + 1) * 128],
                        rhs=xh_slice(i, c8),
                        start=(c8 == 0), stop=(c8 == CT - 1),
                    )
                for d in range(DT):
                    psum_to_sbuf(qt[d][:, i * CW:(i + 1) * CW], qp[d][:])

            def c_mm(cp, tt, i):
                for d in range(DT):
                    nc.tensor.matmul(
                        out=cp[:],
                        lhsT=ktb[d][:, tt * 128:(tt + 1) * 128],
                        rhs=qt[d][:, i * CW:(i + 1) * CW],
                        start=(d == 0), stop=(d == DT - 1),
                    )

            def c_single(tt, i):
                cp = psqk.tile([128, CW], F32, tag="ps", name="cp")
                c_mm(cp, tt, i)
                ob = opool.tile([128, CW], BF16, tag="obs", name="obs")
                psum_to_sbuf(ob[:], cp[:])
                nc.sync.dma_start(out=out[tt][:, i * CW:(i + 1) * CW],
                                  in_=ob[:])

            def c_tile(tt):
                # all 3 blocks of one j-tile -> one [128, NQ*CW] DMA
                cps = []
                for i in range(NQ):
                    cp = psqk.tile([128, CW], F32, tag="ps", name="cp")
                    c_mm(cp, tt, i)
                    cps.append(cp)
                ob = opool.tile([128, NQ * CW], BF16, tag="obf", name="obf")
                for i in range(NQ):
                    psum_to_sbuf(ob[:, i * CW:(i + 1) * CW], cps[i][:])
                nc.sync.dma_start(out=out[tt][:], in_=ob[:])

            def c_pair12(tt):
                # blocks (tt, i=1) and (tt, i=2) -> one contiguous DMA
                cps = []
                for i in (1, 2):
                    cp = psqk.tile([128, CW], F32, tag="ps", name="cp")
                    c_mm(cp, tt, i)
                    cps.append(cp)
                ob = opool.tile([128, 2 * CW], BF16, tag="obp", name="obp")
                psum_to_sbuf(ob[:, 0:CW], cps[0][:])
                psum_to_sbuf(ob[:, CW:2 * CW], cps[1][:])
                nc.sync.dma_start(out=out[tt][:, CW:NQ * CW], in_=ob[:])

            k_half(0)                      # needs wk + xh0
            q_chunk(0)                     # needs only wq (xh0 resident)
            k_half(1)                      # needs xk1
            q_chunk(1)                     # needs xh1
            q_chunk(2)                     # needs xh2
            for tt in range(NK):
                if tt < NK - 1:
                    c_tile(tt)
                else:
                    # pair first, single last: the final DMA then waits on
                    # one cast of one [128,512] block only
                    c_pair12(tt)
                    c_single(tt, 0)
    nc.compile()
    return nc


def kernel(x, idx, Wq, Wk):
    x = np.asarray(x, dtype=np.float32)
    idx = np.asarray(idx).astype(np.int64)
    Wq = np.asarray(Wq, dtype=np.float32)
    Wk = np.asarray(Wk, dtype=np.float32)

    xb = x.astype(BF)
    # xh_all[ch, cin, c8*CW + tin] = x[ch*CW + tin, c8*128 + cin]
    xh_all = np.ascontiguousarray(
        xb.reshape(NCH, CW, CT, 128).transpose(0, 3, 2, 1)
        .reshape(NCH, 128, CT * CW))
    # wq c8-major: wq2[cin, c8*256 + d*128 + col], scaled by 1/256
    wq2 = np.ascontiguousarray(
        (Wq / 256.0).astype(BF).reshape(CT, 128, D).transpose(1, 0, 2)
        .reshape(128, CT * D))
    # wk c8-major: wk2[cin, c8*256 + d*128 + col]
    wk2 = np.ascontiguousarray(
        Wk.astype(BF).reshape(CT, 128, D).transpose(1, 0, 2)
        .reshape(128, CT * D))

    in_maps = []
    for p in range(NCORES):
        chunks, tiles = REGIONS[p]
        m = {"wq": wq2}
        for nm, lo, hi in WK_PIECES:
            m[nm] = np.ascontiguousarray(wk2[:, lo * 256:hi * 256])
        rows = np.concatenate(
            [np.arange(g * 128, (g + 1) * 128) for g in tiles[HK:]])
        # xk[cin, c8*CW + tt*128 + jj] = x[rows[tt*128+jj], c8*128+cin]
        xs = xb[rows]                                  # [HK*128, C]
        xk = np.ascontiguousarray(
            xs.reshape(HK * 128, CT, 128).transpose(2, 1, 0)
            .reshape(128, CT * HK * 128))
        for nm, lo, hi in XK1_PIECES:
            m[nm] = np.ascontiguousarray(xk[:, lo * CW:hi * CW])
        for nm, lo, hi in XH0_PIECES:
            m[nm] = np.ascontiguousarray(
                xh_all[chunks[0]][:, lo * CW:hi * CW])
        for i in (1, 2):
            ch = chunks[i]
            m[f"xh{i}0"] = np.ascontiguousarray(xh_all[ch][:, :4 * CW])
            m[f"xh{i}1"] = np.ascontiguousarray(xh_all[ch][:, 4 * CW:])
        in_maps.append(m)

    nc = _build()
    res = run_bass_kernel_spmd(nc, in_maps, core_ids=list(range(NCORES)))

    # assemble c [T(j), T(t)] in fp32 from the active blocks of each region,
    # apply the causal mask, segment-sum over j -> vocab on the host
    cmat = np.zeros((T, T), np.float32)
    for p in range(NCORES):
        chunks, tiles = REGIONS[p]
        blk = np.asarray(res.results[p]["out"]).astype(np.float32)
        for tt, g in enumerate(tiles):
            for qq, ch in enumerate(chunks):
                if ch >= g // 4:     # causally active block
                    cmat[g * 128:(g + 1) * 128, ch * CW:(ch + 1) * CW] = \
                        blk[tt, :, qq * CW:(qq + 1) * CW]
    jj = np.arange(T)
    cmat *= jj[None, :] >= jj[:, None]      # keep t >= j
    order = np.argsort(idx, kind="stable")
    sidx = idx[order]
    starts = np.flatnonzero(np.r_[True, sidx[1:] != sidx[:-1]])
    red = np.add.reduceat(cmat[order], starts, axis=0)  # [nu, T]
    outf = np.zeros((T, V), np.float32)
    outf[:, sidx[starts]] = red.T
    return outf
